# revision 19
# baseline (speedup 1.0000x reference)
"""TRN2 Bass kernel for nn_DerivNet2D.

Reference computation (per sample x in R^2):
    h1 = W1 @ x + b1;  z1 = tanh(h1)            (1024)
    h2 = W2 @ z1 + b2; z2 = tanh(h2)            (512)
    y  = W3 @ z2 + b3                           (1)
    dy/dx_k = W3 @ (dz2 * (W2 @ (dz1 * W1[:,k])))   k = 1, 2
    returns (y, v1, v2) = (y, dy/dx2, -dy/dx1)

Strategy (v5):
  * Pure data parallel: x split into 8 shards of 8192 samples; weights
    replicated.  SPMD module via run_bass_kernel_spmd.
  * All layouts prepared host-side; no on-chip preprocessing.
  * Reverse-mode gradient: A = w3*(1-z2^2); B = W2.T @ A;
    C = (z1^2-1)*B = -dz1*B;  (y,v) rows = Wyv.T @ [z2-chunks, C-chunks]
    as ONE 12-MM accumulation chain (signs fixed on host).
  * L1 with bias folded into the matmul (K=3: x1, x2, ones row).
  * Everything f32/f32r (same PE rate as bf16) except q2 in bf16.
  * Schedule (per steady-state iteration, PE queue order):
      [fwd(T) 4 chains x 8] [yv-z2(T) x4]
      [bwd(T) chain i -> interleave: L1(T+1) pair after even i,
       yv-C(T) chunk i-1 after chain i] [yv tail] -> copy -> out DMA
    so ACT's tanh chain for tile T+1 runs under bwd(T) and the PE never
    waits on elementwise engines in steady state.
  * PSUM banks: ph1 2 + ph2 3 + pB 2 + pyv 1 = 8.
  * DMA queues: xt + w2t on SP(sync), w2n on ACT(scalar), small preloads
    and output stores on GpSimd so streams never head-block each other.
"""

import numpy as np
from contextlib import ExitStack

import concourse.bacc as bacc
import concourse.mybir as mybir
import concourse.tile as tile
from concourse.bass import ds, ts

F32 = mybir.dt.float32
F32R = mybir.dt.float32r
BF16 = mybir.dt.bfloat16
AF = mybir.ActivationFunctionType
ALU = mybir.AluOpType

NCORES = 8
NX = 65536
NXL = NX // NCORES      # 8192 samples per core
NT = 512                # samples per tile
TILES = NXL // NT       # 16

_CACHE = {}


def build():
    nc = bacc.Bacc(None, target_bir_lowering=False)
    XTb = nc.dram_tensor("XTb", [8, NXL], BF16, kind="ExternalInput")
    W1Tb = nc.dram_tensor("W1Tb", [8, 1024], BF16, kind="ExternalInput")
    W2T = nc.dram_tensor("W2T", [1024, 512], BF16, kind="ExternalInput")
    W2N = nc.dram_tensor("W2N", [512, 1024], BF16, kind="ExternalInput")
    WYV = nc.dram_tensor("WYV", [12, 128, 3], F32, kind="ExternalInput")
    W3S = nc.dram_tensor("W3S", [128, 8], F32, kind="ExternalInput")  # [-w3 | +w3]
    B2S = nc.dram_tensor("B2S", [128, 4], F32, kind="ExternalInput")
    OUT = nc.dram_tensor("out", [3, NXL], F32, kind="ExternalOutput")

    with ExitStack() as ctx:
        tc = ctx.enter_context(tile.TileContext(nc))
        sg = ctx.enter_context(tc.tile_pool(name="sg", bufs=1))
        pxt = ctx.enter_context(tc.tile_pool(name="pxt", bufs=4))
        pz1 = ctx.enter_context(tc.tile_pool(name="pz1", bufs=2))
        pz1b = ctx.enter_context(tc.tile_pool(name="pz1b", bufs=2))
        pq1 = ctx.enter_context(tc.tile_pool(name="pq1", bufs=2))
        pz2 = ctx.enter_context(tc.tile_pool(name="pz2", bufs=2))
        pq2 = ctx.enter_context(tc.tile_pool(name="pq2", bufs=2))
        pA = ctx.enter_context(tc.tile_pool(name="pA", bufs=2))
        pC = ctx.enter_context(tc.tile_pool(name="pC", bufs=2))
        pyo = ctx.enter_context(tc.tile_pool(name="pyo", bufs=2))
        ph1 = ctx.enter_context(tc.tile_pool(name="ph1", bufs=2, space="PSUM"))
        ph2 = ctx.enter_context(tc.tile_pool(name="ph2", bufs=2, space="PSUM"))
        pB = ctx.enter_context(tc.tile_pool(name="pB", bufs=3, space="PSUM"))
        pyv = ctx.enter_context(tc.tile_pool(name="pyv", bufs=1, space="PSUM"))

        # ---- PE warmup: ~24 dummy MMs on a zeroed tile so the HAM
        # clock-gate reaches 2.4 GHz while the weight DMAs stream in ----
        warm = sg.tile([128, NT], BF16)
        nc.vector.memset(warm, 0.0)
        for _ in range(70):
            pw = ph1.tile([128, NT], F32, tag="h1", name="p1")
            nc.tensor.matmul(pw, warm[:, 0:128], warm, start=True, stop=True)

        # ---- preload (pure DMA, split across the three DGE queues) ---
        w1t = sg.tile([40, 1024], BF16)
        nc.sync.dma_start(out=w1t[0:8, :], in_=W1Tb[:, :])
        nc.sync.dma_start(out=w1t[32:40, :], in_=W1Tb[:, :])

        wyv = sg.tile([128, 12, 3], F32R)
        nc.gpsimd.dma_start(
            out=wyv, in_=WYV[:, :, :].rearrange("k p m -> p k m").bitcast(F32R)
        )
        w3s = sg.tile([128, 8], F32)
        nc.gpsimd.dma_start(out=w3s, in_=W3S[:, :])
        b2t = sg.tile([128, 4], F32)
        nc.gpsimd.dma_start(out=b2t, in_=B2S[:, :])

        # ---- software-pipelined main loop ----------------------------
        state = {}

        def emit_xt(T):
            xt = pxt.tile([40, NT], BF16, tag="xt", name="xt")
            nc.sync.dma_start(out=xt[0:8, :], in_=XTb[:, ds(T * NT, NT)])
            nc.sync.dma_start(out=xt[32:40, :], in_=XTb[:, ds(T * NT, NT)])
            state[("xt", T)] = xt

        for _t in range(min(4, TILES)):
            emit_xt(_t)

        # fwd lhsT: w2t[p, j, m] = W2[c*128+m, j*128+p] for m-block c
        w2t = sg.tile([128, 8, 512], BF16)
        for c in range(4):
            for jq in range(4):
                eng = nc.scalar if (c == 0 and jq >= 2) else nc.sync
                eng.dma_start(
                    out=w2t[:, ds(2 * jq, 2), ds(c * 128, 128)],
                    in_=W2T[ds(jq * 256, 256), ds(c * 128, 128)]
                    .rearrange("(j p) m -> p j m", j=2),
                )
        # bwd lhsT: w2n[p, c, m] = W2[c*128+p, m]; split by (m-chunk k, c-half)
        w2n = sg.tile([128, 4, 1024], BF16)
        for k in range(8):
            for ch in range(2):
                nc.scalar.dma_start(
                    out=w2n[:, ds(2 * ch, 2), ds(k * 128, 128)],
                    in_=W2N[ds(ch * 256, 256), ds(k * 128, 128)]
                    .rearrange("(c p) m -> p c m", c=2),
                )

        def emit_l1_pair(T, c0):
            """Two L1 chunk MMs + their tanhs (h1 banks drained by ACT)."""
            xt = state[("xt", T)]
            if c0 == 0:
                state[("z1", T)] = pz1.tile(
                    [128, 8, NT], F32R, tag="z1", name="z1r"
                )
                state[("z1b", T)] = pz1b.tile(
                    [128, 8, NT], BF16, tag="z1b", name="z1b"
                )
            z1r = state[("z1", T)]
            z1b = state[("z1b", T)]
            for g, c1 in ((0, c0), (32, c0 + 1)):
                p1 = ph1.tile([128, NT], F32, tag="h1", name="p1")
                nc.tensor.matmul(
                    p1,
                    w1t[g : g + 8, ts(c1, 128)],
                    xt[g : g + 8, :],
                    start=True, stop=True,
                    tile_position=(g, 0),
                )
                nc.scalar.activation(z1r[:, c1, :], p1, AF.Tanh)
            nc.gpsimd.dma_start(
                out=z1b[:, ds(c0, 2), :], in_=z1r[:, ds(c0, 2), :].bitcast(F32)
            )
            if c0 == 6:
                state[T] = state.pop(("z1", T))

        for T in range(TILES + 1):
            if 4 <= T + 3 < TILES:
                emit_xt(T + 3)
            if T == 0:
                for c0 in (0, 2, 4, 6):
                    emit_l1_pair(0, c0)
                continue

            # ---------------- rest of tile T-1 ------------------------
            Tm = T - 1
            sl = ds(Tm * NT, NT)
            z1r = state.pop(Tm)
            z1b = state.pop(("z1b", Tm))
            state.pop(("xt", Tm), None)

            # q1 = z1^2 on the (otherwise idle) GpSimd engine, two halves
            q1 = pq1.tile([128, 8, NT], F32, tag="q1", name="q1")
            for h in range(2):
                nc.gpsimd.tensor_mul(
                    q1[:, ds(4 * h, 4), :],
                    z1r[:, ds(4 * h, 4), :].bitcast(F32),
                    z1r[:, ds(4 * h, 4), :].bitcast(F32),
                )

            # fwd: h2 = W2 @ z1 + b2; per chunk: tanh, q2 = z2^2,
            # A = q2*(-w3) + w3
            z2 = pz2.tile([128, 4, NT], F32R, tag="z2", name="z2")
            q2 = pq2.tile([128, 4, NT], BF16, tag="q2", name="q2")
            A = pA.tile([128, 4, NT], BF16, tag="A", name="A")
            for c in range(4):
                p2 = ph2.tile([128, NT], F32, tag="h2", name="p2")
                for j in range(8):
                    nc.tensor.matmul(
                        p2,
                        w2t[:, j, ds(c * 128, 128)],
                        z1b[:, j, :],
                        start=(j == 0), stop=(j == 7),
                    )
                nc.scalar.activation(
                    z2[:, c, :], p2, AF.Tanh, bias=b2t[:, c : c + 1]
                )
                nc.vector.tensor_mul(
                    q2[:, c, :], z2[:, c, :].bitcast(F32), z2[:, c, :].bitcast(F32)
                )
                nc.scalar.activation(
                    A[:, c, :], q2[:, c, :], AF.Identity,
                    bias=w3s[:, 4 + c : 5 + c], scale=w3s[:, c : c + 1],
                )

            # yv chain start: 4 z2-chunk MMs (chunks 8..11 of Wyv)
            pyvt = pyv.tile([3, NT], F32, tag="yv", name="pyvt")
            for k in range(4):
                nc.tensor.matmul(
                    pyvt[0:3, :], wyv[:, 8 + k, :], z2[:, k, :],
                    start=(k == 0), stop=False, skip_group_check=True,
                )

            # bwd chains with L1(T) pairs and yv-C chunks interleaved
            C = pC.tile([128, 8, NT], F32R, tag="C", name="C")
            for i in range(8):
                pb = pB.tile([128, NT], F32, tag="B", name="pb")
                for c in range(4):
                    nc.tensor.matmul(
                        pb,
                        w2n[:, c, ds(i * 128, 128)],
                        A[:, c, :],
                        start=(c == 0), stop=(c == 3),
                    )
                nc.vector.scalar_tensor_tensor(
                    out=C[:, i, :], in0=q1[:, i, :], scalar=1.0, in1=pb,
                    op0=ALU.subtract, op1=ALU.mult,
                )
                if i % 2 == 0 and T < TILES:
                    emit_l1_pair(T, i)
            for k in range(8):
                nc.tensor.matmul(
                    pyvt[0:3, :], wyv[:, k, :], C[:, k, :],
                    start=False, stop=(k == 7), skip_group_check=True,
                )

            yvs = pyo.tile([3, NT], F32, tag="yvs", name="yvs")
            nc.vector.tensor_copy(yvs, pyvt)
            nc.gpsimd.dma_start(out=OUT[:, sl], in_=yvs[0:3, :])

    nc.compile()
    return nc


def prep_inputs(x_shard, W1, b1, W2, b2, W3, b3):
    """Host-side layout prep for one core's shard."""
    import ml_dtypes

    f32 = np.float32
    bf16 = ml_dtypes.bfloat16
    # L1 in bf16 with hi/lo splitting: K=8 rows
    #   lhsT: [w1a_hi, w1a_hi, w1a_lo, w1b_hi, w1b_hi, w1b_lo, b1_hi, b1_lo]
    #   rhs:  [x1hi,   x1lo,   x1hi,   x2hi,   x2lo,   x2hi,   1,     1   ]
    x1 = x_shard[:, 0].astype(f32)
    x2 = x_shard[:, 1].astype(f32)
    x1hi = x1.astype(bf16)
    x1lo = (x1 - x1hi.astype(f32)).astype(bf16)
    x2hi = x2.astype(bf16)
    x2lo = (x2 - x2hi.astype(f32)).astype(bf16)
    one = np.ones(NXL, bf16)
    xtb = np.stack([x1hi, x1lo, x1hi, x2hi, x2lo, x2hi, one, one])
    w1a = W1[:, 0].astype(f32)
    w1b = W1[:, 1].astype(f32)
    w1a_hi = w1a.astype(bf16)
    w1a_lo = (w1a - w1a_hi.astype(f32)).astype(bf16)
    w1b_hi = w1b.astype(bf16)
    w1b_lo = (w1b - w1b_hi.astype(f32)).astype(bf16)
    b1hi = b1.astype(bf16)
    b1lo = (b1.astype(f32) - b1hi.astype(f32)).astype(bf16)
    w1tb = np.stack([w1a_hi, w1a_hi, w1a_lo, w1b_hi, w1b_hi, w1b_lo, b1hi, b1lo])
    wyv = np.zeros((12, 128, 3), f32)
    for i in range(8):
        blk = W1[i * 128 : (i + 1) * 128]
        wyv[i, :, 0] = blk[:, 1]
        wyv[i, :, 1] = blk[:, 0]
    for c in range(4):
        wyv[8 + c, :, 2] = W3[0, c * 128 : (c + 1) * 128]
    w3s = np.empty((128, 8), f32)
    w3r = W3[0].reshape(4, 128).T  # [p, c]
    w3s[:, 0:4] = -w3r
    w3s[:, 4:8] = w3r
    b2s = np.ascontiguousarray(b2.reshape(4, 128).T)
    return {
        "XTb": np.ascontiguousarray(xtb),
        "W1Tb": np.ascontiguousarray(w1tb),
        "W2T": np.ascontiguousarray(W2.T).astype(bf16),
        "W2N": np.ascontiguousarray(W2).astype(bf16),
        "WYV": wyv,
        "W3S": np.ascontiguousarray(w3s),
        "B2S": np.ascontiguousarray(b2s.astype(f32)),
    }


def postprocess(o, b3):
    """o: [3, NXL] -> (y, v1, v2) for the shard."""
    v1 = -o[0]
    v2 = o[1]
    y = o[2] + b3[0]
    return y, v1, v2


def kernel(x, W1, b1, W2, b2, W3, b3):
    from concourse.bass_utils import run_bass_kernel_spmd

    if "nc" not in _CACHE:
        _CACHE["nc"] = build()
    nc = _CACHE["nc"]

    x = np.asarray(x, dtype=np.float32)
    W1 = np.asarray(W1, dtype=np.float32)
    b1 = np.asarray(b1, dtype=np.float32)
    W2 = np.asarray(W2, dtype=np.float32)
    b2 = np.asarray(b2, dtype=np.float32)
    W3 = np.asarray(W3, dtype=np.float32)
    b3 = np.asarray(b3, dtype=np.float32)

    shards = np.split(x, NCORES, axis=0)
    in_maps = [
        prep_inputs(shards[c], W1, b1, W2, b2, W3, b3) for c in range(NCORES)
    ]
    _CACHE["in_maps"] = in_maps

    res = run_bass_kernel_spmd(nc, in_maps, core_ids=list(range(NCORES)))
    ys, v1s, v2s = [], [], []
    for c in range(NCORES):
        y, v1, v2 = postprocess(res.results[c]["out"], b3)
        ys.append(y)
        v1s.append(v1)
        v2s.append(v2)
    y = np.concatenate(ys).reshape(NX, 1).astype(np.float32)
    v1 = np.concatenate(v1s).reshape(NX, 1).astype(np.float32)
    v2 = np.concatenate(v2s).reshape(NX, 1).astype(np.float32)
    return (y, v1, v2)
